# revision 56
# baseline (speedup 1.0000x reference)
"""Trainium2 Bass kernel: 3D-RoPE multi-head attention (B=4,N=2048,DIM=1536,H=16,DH=96).

Sharding: 8 cores = (batch b = c//2) x (head half hh = c%2, 8 heads each).
Each core computes, for its batch and its 8 heads:
  - merged Q/K/V projection in token layout (full 128-wide contraction),
    RoPE on Q/K via shifted free-dim multiplies with (+-)sin tables, then
    PE transposes into the [dh, token] layout attention needs; everything
    stays SBUF-resident
  - attention per (query-tile, head) unit: score groups double-buffered
    against the scalar-engine exp; softmax denominator via an appended
    ones-column in V; normalization + row-split output projection of the
    previous query tile are interleaved as PE filler between score groups
  - host sums the two partial output projections per batch. Bias is fed to
    the hh==0 core only (hh==1 gets zeros).
All matmul inputs are bf16; accumulation is fp32 in PSUM. Inputs are
host-packed into tile layouts so all large DMAs are contiguous.
"""

import sys

if "/opt/trn_rl_repo" not in sys.path:
    sys.path.insert(0, "/opt/trn_rl_repo")

import numpy as np

import concourse.bass as bass
import concourse.mybir as mybir
import concourse.tile as tile
from concourse import bacc
from concourse.bass_utils import run_bass_kernel_spmd

B, N, DIM, H, DH = 4, 2048, 1536, 16, 96
HC = H // 2          # heads per core
HD = HC * DH         # 768 projected cols per core
SCALE = DH ** -0.5
KT = DIM // 128      # 12 contraction tiles
TT = 512             # query tile
NMT = N // 128       # 16 key chunks
NQT = N // TT        # 4 query tiles
F32 = mybir.dt.float32
F32R = mybir.dt.float32r
BF16 = mybir.dt.bfloat16
IN_DT = BF16
import ml_dtypes
IN_NP = ml_dtypes.bfloat16
AF = mybir.ActivationFunctionType
HALF_PI = float(np.pi / 2)


def _emit(ctx, tc, io):
    nc = tc.nc
    xP, fN, ident, Wall, WoC, boutC, out = io

    persist = ctx.enter_context(tc.tile_pool(name="persist", bufs=1))

    # ---- constants ------------------------------------------------------
    ones1f = persist.tile([1, DH], F32, tag="ones1f")
    nc.vector.memset(ones1f, 1.0)
    ones1 = persist.tile([1, DH], F32R, tag="ones1")
    nc.scalar.copy(out=ones1, in_=ones1f)
    halfpi = persist.tile([128, 1], F32, tag="halfpi")
    nc.vector.memset(halfpi, HALF_PI)

    ident_sb = persist.tile([128, 128], IN_DT, tag="ident")
    cosT = persist.tile([128, NMT, DH], IN_DT, tag="cosT")
    sinT = persist.tile([128, NMT, DH], IN_DT, tag="sinT")
    nsinT = persist.tile([128, NMT, DH], IN_DT, tag="nsinT")
    KTs = [persist.tile([DH, N], IN_DT, tag=f"kt{h}", name=f"kt{h}")
           for h in range(HC)]
    QTs = [persist.tile([DH, N], IN_DT, tag=f"qt{h}", name=f"qt{h}")
           for h in range(HC)]
    Vt = persist.tile([128, NMT, HC, DH + 1], IN_DT, tag="vt")
    nc.vector.memset(Vt[:, :, :, DH:DH + 1], 1.0)

    # ---- merged QKV projection in token layout --------------------------
    # One pass over 16 token tiles; per tile 6 column chunks of 384
    # (Q heads 0-3, Q 4-7, K 0-3, K 4-7, V 0-3, V 4-7), all with the full
    # 128-wide contraction. Q/K get RoPE via free-dim strided ops, then a
    # PE transpose into the [dh, token] layout attention wants. xP/Wall/fN
    # come host-packed in tile layout so the DMAs are fully contiguous.

    def bc_heads(base):
        """broadcast a [128, ...] AP across 4 heads via a 0-stride dim."""
        return bass.AP(tensor=base.tensor, offset=base.offset,
                       ap=[list(base.ap[0]), [0, 4]]
                          + [list(p) for p in base.ap[1:]])

    with (
        tc.tile_pool(name="pw", bufs=1) as pw,
        tc.tile_pool(name="pxt", bufs=3) as pxt,
        tc.tile_pool(name="ptrg", bufs=1) as ptrg,
        tc.tile_pool(name="prp", bufs=2) as prp,
        tc.tile_pool(name="psp", bufs=3, space="PSUM") as psp,
        tc.tile_pool(name="pstp", bufs=3, space="PSUM") as pstp,
    ):
        W_sb = pw.tile([128, 6, KT, 384], IN_DT, tag="w")
        nc.sync.dma_start(out=W_sb[:, 0], in_=Wall[:, 0])

        def xt_dma(t):
            xt = pxt.tile([128, KT, 128], IN_DT, tag="xt", name=f"xt{t}")
            nc.sync.dma_start(out=xt, in_=xP[:, t])
            return xt
        xt_cur = xt_dma(0)
        fN_sb = ptrg.tile([128, NMT, DH], F32, tag="fN")
        nc.sync.dma_start(out=fN_sb, in_=fN)
        nc.sync.dma_start(out=ident_sb, in_=ident)
        nc.scalar.activation(out=sinT, in_=fN_sb, func=AF.Sin)
        nc.scalar.activation(out=cosT, in_=fN_sb, func=AF.Sin, bias=halfpi)
        nc.scalar.activation(out=nsinT, in_=fN_sb, func=AF.Sin, scale=-1.0)
        for c in range(1, 6):
            nc.sync.dma_start(out=W_sb[:, c], in_=Wall[:, c])

        def half16(tile, t, half):
            """[128, 3, 16] view of one rotate-half of a [128, DH] slice."""
            base = tile[:, t, :]
            return bass.AP(tensor=base.tensor, offset=base.offset + 16 * half,
                           ap=[list(base.ap[0]), [32, 3], [1, 16]])

        for t in range(NMT):
            xt = xt_cur
            if t + 1 < NMT:
                xt_cur = xt_dma(t + 1)
            for c in range(6):
                pp = psp.tile([128, 4, 3, 32], F32, tag="pp", name="pp")
                for k in range(KT):
                    nc.tensor.matmul(
                        out=pp, lhsT=xt[:, k, :],
                        rhs=W_sb[:, c, k, :],
                        start=(k == 0), stop=(k == KT - 1),
                    )
                if c >= 4:
                    j = c - 4
                    nc.scalar.copy(out=Vt[:, t, 4 * j:4 * j + 4, 0:DH],
                                   in_=pp)
                    continue
                # RoPE: dest = pp*cos + rot(pp)*sin; the rotate-half is
                # folded into two shifted multiplies with a negated sin
                u = prp.tile([128, 4, DH], F32, tag="u", name="u")
                nc.vector.tensor_mul(out=u, in0=pp, in1=bc_heads(cosT[:, t, :]))
                w = prp.tile([128, 4, 3, 32], F32, tag="wv", name="wv")
                nc.vector.tensor_mul(out=w[:, :, :, 0:16],
                                     in0=pp[:, :, :, 16:32],
                                     in1=bc_heads(half16(nsinT, t, 0)))
                nc.vector.tensor_mul(out=w[:, :, :, 16:32],
                                     in0=pp[:, :, :, 0:16],
                                     in1=bc_heads(half16(sinT, t, 1)))
                qk = prp.tile([128, 4, DH], IN_DT, tag="qk", name="qk")
                nc.vector.tensor_add(out=qk, in0=u, in1=w)
                dest = QTs if c < 2 else KTs
                h0 = (c % 2) * 4
                for hh in range(4):
                    tp = pstp.tile([DH, 128], IN_DT, tag="tp", name="tp")
                    nc.tensor.transpose(out=tp, in_=qk[:, hh, :],
                                        identity=ident_sb)
                    dst = dest[h0 + hh][:, t * 128:(t + 1) * 128]
                    if hh % 2 == 0:
                        nc.vector.tensor_copy(out=dst, in_=tp)
                    else:
                        nc.scalar.copy(out=dst, in_=tp)

    # ---- attention + output projection ----------------------------------
    NG = 2          # key chunks per score group (one exp instruction each)
    NGRP = NMT // NG
    with (
        tc.tile_pool(name="patt", bufs=1) as pat,
        tc.tile_pool(name="pex", bufs=2) as pex,
        tc.tile_pool(name="pdt", bufs=2) as pdt,
        tc.tile_pool(name="posb", bufs=2) as posb,
        tc.tile_pool(name="psc", bufs=2, space="PSUM") as psc,
        tc.tile_pool(name="psho", bufs=2, space="PSUM") as psho,
        tc.tile_pool(name="pse", bufs=2, space="PSUM") as pse,
    ):
        WoSB = pat.tile([DH, HC, DIM], IN_DT, tag="wo")
        nc.sync.dma_start(out=WoSB, in_=WoC.rearrange("(h p) c -> p h c", p=DH))
        bias_sb = pat.tile([128, DIM], F32, tag="bias")
        bout_bc = bass.AP(tensor=boutC.tensor, offset=boutC.offset,
                          ap=[[0, 128]] + [list(p) for p in boutC.ap])
        nc.sync.dma_start(out=bias_sb, in_=bout_bc)
        hoU = pat.tile([DH, 2, HC, TT], IN_DT, tag="hoU")
        hoT = pat.tile([DH, HC, TT], IN_DT, tag="hoT")
        dn = pat.tile([HC, 2, TT], F32, tag="dn")
        dnf = pat.tile([HC, 2, TT], F32R, tag="dnf")
        rcp = pat.tile([1, 2, HC, TT], F32R, tag="rcp")

        # finish work for the previous query tile is emitted piecewise
        # between score/AV groups so the PE fills exp-wait bubbles: 57
        # fine-grained pieces per query tile over ~64 pop slots, one piece
        # per slot so every score group gets guaranteed PE filler work.
        filler = []

        def pop_filler(force=False):
            if filler:
                filler.pop(0)()

        def attn_unit(qt, h):
            """scores -> exp -> AV for (query tile qt, head h).

            Group-level software pipeline: sc(g) runs while exp(g-1) is on
            the scalar engine; av(g-1) follows sc(g)."""
            par = qt % 2
            qsl = slice(qt * TT, (qt + 1) * TT)
            ho_ps = psho.tile([DH + 1, TT], F32, tag="hops", name="hops")
            ex = [None, None]

            def sc_group(g):
                scp = psc.tile([128, NG, TT], F32, tag="scps", name="scps")
                for j in range(NG):
                    p = NG * g + j
                    nc.tensor.matmul(
                        out=scp[:, j, :],
                        lhsT=KTs[h][:, p * 128:(p + 1) * 128],
                        rhs=QTs[h][:, qsl], start=True, stop=True,
                    )
                ex[g % 2] = pex.tile([128, NG, TT], IN_DT, tag="ex", name="ex")
                nc.scalar.activation(out=ex[g % 2], in_=scp, func=AF.Exp,
                                     scale=SCALE)

            def av_group(g):
                for j in range(NG):
                    p = NG * g + j
                    nc.tensor.matmul(
                        out=ho_ps, lhsT=Vt[:, p, h, :], rhs=ex[g % 2][:, j, :],
                        start=(p == 0), stop=(p == NMT - 1),
                    )

            sc_group(0)
            for g in range(1, NGRP):
                sc_group(g)
                av_group(g - 1)
                pop_filler()
            av_group(NGRP - 1)
            # stash unnormalized numerator + denominator row (the
            # denominator goes via same-partition copy + DMA shuffle);
            # scalar handles the last tile so vector is free for the tail
            dt = pdt.tile([DH + 1, TT], F32, tag="dt", name="dt")
            if qt == NQT - 1:
                nc.scalar.copy(out=hoU[:, par, h, :], in_=ho_ps[0:DH, :])
                nc.scalar.copy(out=dt[DH:DH + 1, :], in_=ho_ps[DH:DH + 1, :])
            else:
                nc.vector.tensor_copy(out=hoU[:, par, h, :],
                                      in_=ho_ps[0:DH, :])
                nc.vector.tensor_copy(out=dt[DH:DH + 1, :],
                                      in_=ho_ps[DH:DH + 1, :])
            nc.sync.dma_start(out=dn[h:h + 1, par, :], in_=dt[DH:DH + 1, :])

        def fin_recip(qt):
            par = qt % 2
            with nc.allow_low_precision(reason="f32r is bitwise f32"):
                nc.vector.reciprocal(out=dnf[:, par, :], in_=dn[:, par, :])
            for h in range(HC):
                nc.sync.dma_start(out=rcp[0:1, par, h, :],
                                  in_=dnf[h:h + 1, par, :])

        def fin_norm(qt, h):
            par = qt % 2
            bc = psho.tile([DH + 1, TT], F32, tag="hops", name="bc")
            nc.tensor.matmul(out=bc[0:DH, :], lhsT=ones1,
                             rhs=rcp[0:1, par, h, :], start=True, stop=True)
            nc.vector.tensor_mul(out=hoT[:, h, :], in0=hoU[:, par, h, :],
                                 in1=bc[0:DH, :])

        def fin_chain_part(qt, tt_i, et, part, holder):
            """two matmuls of the 8-head out-proj chain; part 3 finishes."""
            if part == 0:
                holder[0] = pse.tile([128, TT], F32, tag="eps", name="eps")
            eps = holder[0]
            for h in (2 * part, 2 * part + 1):
                nc.tensor.matmul(
                    out=eps,
                    lhsT=hoT[:, h, tt_i * 128:(tt_i + 1) * 128],
                    rhs=WoSB[:, h, et * TT:(et + 1) * TT],
                    start=(h == 0), stop=(h == HC - 1),
                )
            if part == 3:
                osb = posb.tile([128, TT], F32, tag="osb", name="osb")
                nc.vector.tensor_add(
                    out=osb, in0=eps, in1=bias_sb[:, et * TT:(et + 1) * TT]
                )
                nc.sync.dma_start(out=out[(qt * 4 + tt_i) * 3 + et], in_=osb)

        def queue_finish(qt):
            filler.append(lambda qt=qt: fin_recip(qt))
            for h in range(HC):
                filler.append(lambda qt=qt, h=h: fin_norm(qt, h))
            for tt_i in range(TT // 128):
                for et in range(DIM // TT):
                    holder = [None]
                    for part in range(4):
                        filler.append(
                            lambda qt=qt, t=tt_i, e=et, p=part, hd=holder:
                            fin_chain_part(qt, t, e, p, hd))

        for qt in range(NQT):
            for h in range(HC):
                attn_unit(qt, h)
                pop_filler()
            queue_finish(qt)
        while filler:
            pop_filler(force=True)


def build():
    from contextlib import ExitStack

    nc = bacc.Bacc("TRN2", target_bir_lowering=False, debug=False)
    xP_p = nc.declare_dram_parameter("xP", [128, NMT, KT, 128], IN_DT,
                                     isOutput=False)
    fN_p = nc.declare_dram_parameter("fN", [128, NMT, DH], F32, isOutput=False)
    id_p = nc.declare_dram_parameter("ident", [128, 128], IN_DT, isOutput=False)
    Wall_p = nc.declare_dram_parameter("Wall", [128, 6, KT, 384], IN_DT,
                                       isOutput=False)
    WoC_p = nc.declare_dram_parameter("WoC", [HD, DIM], IN_DT, isOutput=False)
    bout_p = nc.declare_dram_parameter("boutC", [DIM], F32, isOutput=False)
    out = nc.declare_dram_parameter("out", [NQT * 4 * 3, 128, TT], F32,
                                    isOutput=True)
    io = tuple(
        t[:] for t in (xP_p, fN_p, id_p, Wall_p, WoC_p, bout_p, out)
    )
    with ExitStack() as ctx:
        tc = ctx.enter_context(tile.TileContext(nc))
        _emit(ctx, tc, io)
    nc.finalize()
    return nc


def make_in_maps(x, f1, f2, f3, Wqkv, Wout, bout):
    x = np.asarray(x, np.float32)
    fcat = np.concatenate(
        [np.asarray(f1, np.float32), np.asarray(f2, np.float32),
         np.asarray(f3, np.float32)], axis=1,
    )  # [N, DH]
    # fN packed to [p, t, c]: fN[p,t,c] = fcat[t*128+p, c]
    fN_np = np.ascontiguousarray(
        fcat.reshape(NMT, 128, DH).transpose(1, 0, 2))
    ident_np = np.eye(128, dtype=np.float32).astype(IN_NP)
    Wqkv = np.asarray(Wqkv, np.float32)
    Wout = np.asarray(Wout, np.float32)
    bout = np.ascontiguousarray(np.asarray(bout, np.float32))
    zbias = np.zeros_like(bout)
    # x packed to [p, t, k, n]: xP[p,t,k,n] = x[b, t*128+n, k*128+p]
    xPs = [np.ascontiguousarray(
        x[b].reshape(NMT, 128, KT, 128).transpose(3, 0, 2, 1)).astype(IN_NP)
        for b in range(B)]
    Wslice = []
    for hh in range(2):
        cs = hh * HD
        wall = np.concatenate(
            [Wqkv[:, cs:cs + HD],
             Wqkv[:, H * DH + cs:H * DH + cs + HD],
             Wqkv[:, 2 * H * DH + cs:2 * H * DH + cs + HD]], axis=1,
        )  # [DIM, 2304]
        # packed to [p, c, k, j]: Wall[p,c,k,j] = wall[k*128+p, c*384+j]
        wall_p = np.ascontiguousarray(
            wall.reshape(KT, 128, 6, 384).transpose(1, 2, 0, 3)).astype(IN_NP)
        Wslice.append(dict(
            Wall=wall_p,
            WoC=np.ascontiguousarray(Wout[cs:cs + HD, :]).astype(IN_NP),
        ))
    in_maps = []
    for c in range(8):
        b, hh = divmod(c, 2)
        in_maps.append(dict(
            xP=xPs[b], fN=fN_np, ident=ident_np,
            boutC=bout if hh == 0 else zbias,
            **Wslice[hh],
        ))
    return in_maps


_NC_CACHE = None


def kernel(x, f1, f2, f3, Wqkv, Wout, bout, _trace=False):
    global _NC_CACHE
    if _NC_CACHE is None:
        _NC_CACHE = build()
    nc = _NC_CACHE
    in_maps = make_in_maps(x, f1, f2, f3, Wqkv, Wout, bout)
    res = run_bass_kernel_spmd(nc, in_maps, list(range(8)), trace=_trace)
    out = np.empty((B, N, DIM), np.float32)
    for b in range(B):
        blk = res.results[2 * b]["out"] + res.results[2 * b + 1]["out"]
        # [qt*4+tt, et, 128, 512] blocks -> [2048, 1536]
        out[b] = (blk.reshape(NMT, 3, 128, TT).transpose(0, 2, 1, 3)
                  .reshape(N, DIM))
    if _trace:
        return out, res
    return out


# revision 65
# speedup vs baseline: 1.0321x; 1.0321x over previous
"""Trainium2 Bass kernel: 3D-RoPE multi-head attention (B=4,N=2048,DIM=1536,H=16,DH=96).

Sharding: 8 cores = (batch b = c//2) x (head half hh = c%2, 8 heads each).
Each core computes, for its batch and its 8 heads:
  - merged Q/K/V projection in token layout (full 128-wide contraction),
    RoPE on Q/K via shifted free-dim multiplies with (+-)sin tables, then
    PE transposes into the [dh, token] layout attention needs; everything
    stays SBUF-resident
  - attention per (query-tile, head) unit: score groups double-buffered
    against the scalar-engine exp; softmax denominator via an appended
    ones-column in V; normalization + row-split output projection of the
    previous query tile are interleaved as PE filler between score groups
  - host sums the two partial output projections per batch. Bias is fed to
    the hh==0 core only (hh==1 gets zeros).
All matmul inputs are bf16; accumulation is fp32 in PSUM. Inputs are
host-packed into tile layouts so all large DMAs are contiguous.
"""

import sys

if "/opt/trn_rl_repo" not in sys.path:
    sys.path.insert(0, "/opt/trn_rl_repo")

import numpy as np

import concourse.bass as bass
import concourse.mybir as mybir
import concourse.tile as tile
from concourse import bacc
from concourse.bass_utils import run_bass_kernel_spmd

B, N, DIM, H, DH = 4, 2048, 1536, 16, 96
HC = H // 2          # heads per core
HD = HC * DH         # 768 projected cols per core
SCALE = DH ** -0.5
KT = DIM // 128      # 12 contraction tiles
TT = 512             # query tile
NMT = N // 128       # 16 key chunks
NQT = N // TT        # 4 query tiles
F32 = mybir.dt.float32
F32R = mybir.dt.float32r
BF16 = mybir.dt.bfloat16
IN_DT = BF16
import ml_dtypes
IN_NP = ml_dtypes.bfloat16
AF = mybir.ActivationFunctionType
HALF_PI = float(np.pi / 2)


def _emit(ctx, tc, io):
    nc = tc.nc
    xP, fN, ident, Wall, WoC, boutC, out = io

    persist = ctx.enter_context(tc.tile_pool(name="persist", bufs=1))

    # ---- constants ------------------------------------------------------
    ones1f = persist.tile([1, DH], F32, tag="ones1f")
    nc.vector.memset(ones1f, 1.0)
    ones1 = persist.tile([1, DH], F32R, tag="ones1")
    nc.scalar.copy(out=ones1, in_=ones1f)
    halfpi = persist.tile([128, 1], F32, tag="halfpi")
    nc.vector.memset(halfpi, HALF_PI)

    ident_sb = persist.tile([128, 128], IN_DT, tag="ident")
    cosT = persist.tile([128, NMT, DH], IN_DT, tag="cosT")
    sinT = persist.tile([128, NMT, DH], IN_DT, tag="sinT")
    nsinT = persist.tile([128, NMT, DH], IN_DT, tag="nsinT")
    KTs = [persist.tile([DH, N], IN_DT, tag=f"kt{h}", name=f"kt{h}")
           for h in range(HC)]
    QTs = [persist.tile([DH, N], IN_DT, tag=f"qt{h}", name=f"qt{h}")
           for h in range(HC)]
    Vt = persist.tile([128, NMT, HC, DH + 1], IN_DT, tag="vt")
    nc.vector.memset(Vt[:, :, :, DH:DH + 1], 1.0)

    # ---- merged QKV projection in token layout --------------------------
    # One pass over 16 token tiles; per tile 6 column chunks of 384
    # (Q heads 0-3, Q 4-7, K 0-3, K 4-7, V 0-3, V 4-7), all with the full
    # 128-wide contraction. Q/K get RoPE via free-dim strided ops, then a
    # PE transpose into the [dh, token] layout attention wants. xP/Wall/fN
    # come host-packed in tile layout so the DMAs are fully contiguous.

    def bc_heads(base):
        """broadcast a [128, ...] AP across 4 heads via a 0-stride dim."""
        return bass.AP(tensor=base.tensor, offset=base.offset,
                       ap=[list(base.ap[0]), [0, 4]]
                          + [list(p) for p in base.ap[1:]])

    with (
        tc.tile_pool(name="pw", bufs=1) as pw,
        tc.tile_pool(name="pxt", bufs=3) as pxt,
        tc.tile_pool(name="ptrg", bufs=1) as ptrg,
        tc.tile_pool(name="prp", bufs=2) as prp,
        tc.tile_pool(name="psp", bufs=3, space="PSUM") as psp,
        tc.tile_pool(name="pstp", bufs=3, space="PSUM") as pstp,
    ):
        W_sb = pw.tile([128, 6, KT, 384], IN_DT, tag="w")
        nc.sync.dma_start(out=W_sb[:, 0], in_=Wall[:, 0])

        def xt_dma(t):
            xt = pxt.tile([128, KT, 128], IN_DT, tag="xt", name=f"xt{t}")
            nc.sync.dma_start(out=xt, in_=xP[:, t])
            return xt
        xt_cur = xt_dma(0)
        fN_sb = ptrg.tile([128, NMT, DH], F32, tag="fN")
        nc.sync.dma_start(out=fN_sb, in_=fN)
        nc.sync.dma_start(out=ident_sb, in_=ident)
        nc.scalar.activation(out=sinT, in_=fN_sb, func=AF.Sin)
        nc.scalar.activation(out=cosT, in_=fN_sb, func=AF.Sin, bias=halfpi)
        nc.scalar.activation(out=nsinT, in_=fN_sb, func=AF.Sin, scale=-1.0)
        for c in range(1, 6):
            nc.sync.dma_start(out=W_sb[:, c], in_=Wall[:, c])

        def half16(tile, t, half):
            """[128, 3, 16] view of one rotate-half of a [128, DH] slice."""
            base = tile[:, t, :]
            return bass.AP(tensor=base.tensor, offset=base.offset + 16 * half,
                           ap=[list(base.ap[0]), [32, 3], [1, 16]])

        for t in range(NMT):
            xt = xt_cur
            if t + 1 < NMT:
                xt_cur = xt_dma(t + 1)
            for c in range(6):
                pp = psp.tile([128, 4, 3, 32], F32, tag="pp", name="pp")
                for k in range(KT):
                    nc.tensor.matmul(
                        out=pp, lhsT=xt[:, k, :],
                        rhs=W_sb[:, c, k, :],
                        start=(k == 0), stop=(k == KT - 1),
                    )
                if c >= 4:
                    j = c - 4
                    nc.scalar.copy(out=Vt[:, t, 4 * j:4 * j + 4, 0:DH],
                                   in_=pp)
                    continue
                # RoPE: dest = pp*cos + rot(pp)*sin; the rotate-half is
                # folded into two shifted multiplies with a negated sin
                u = prp.tile([128, 4, DH], F32, tag="u", name="u")
                nc.vector.tensor_mul(out=u, in0=pp, in1=bc_heads(cosT[:, t, :]))
                w = prp.tile([128, 4, 3, 32], F32, tag="wv", name="wv")
                nc.vector.tensor_mul(out=w[:, :, :, 0:16],
                                     in0=pp[:, :, :, 16:32],
                                     in1=bc_heads(half16(nsinT, t, 0)))
                nc.vector.tensor_mul(out=w[:, :, :, 16:32],
                                     in0=pp[:, :, :, 0:16],
                                     in1=bc_heads(half16(sinT, t, 1)))
                qk = prp.tile([128, 4, DH], IN_DT, tag="qk", name="qk")
                nc.vector.tensor_add(out=qk, in0=u, in1=w)
                dest = QTs if c < 2 else KTs
                h0 = (c % 2) * 4
                for hh in range(4):
                    tp = pstp.tile([DH, 128], IN_DT, tag="tp", name="tp")
                    nc.tensor.transpose(out=tp, in_=qk[:, hh, :],
                                        identity=ident_sb)
                    dst = dest[h0 + hh][:, t * 128:(t + 1) * 128]
                    if hh % 2 == 0:
                        nc.vector.tensor_copy(out=dst, in_=tp)
                    else:
                        nc.scalar.copy(out=dst, in_=tp)

    # ---- attention + output projection ----------------------------------
    NG = 2          # key chunks per score group (one exp instruction each)
    NGRP = NMT // NG
    with (
        tc.tile_pool(name="patt", bufs=1) as pat,
        tc.tile_pool(name="pex", bufs=2) as pex,
        tc.tile_pool(name="pdt", bufs=2) as pdt,
        tc.tile_pool(name="posb", bufs=2) as posb,
        tc.tile_pool(name="psc", bufs=2, space="PSUM") as psc,
        tc.tile_pool(name="psho", bufs=2, space="PSUM") as psho,
        tc.tile_pool(name="pse", bufs=2, space="PSUM") as pse,
    ):
        WoSB = pat.tile([DH, HC, DIM], IN_DT, tag="wo")
        nc.sync.dma_start(out=WoSB, in_=WoC.rearrange("(h p) c -> p h c", p=DH))
        bias_sb = pat.tile([128, DIM], F32, tag="bias")
        bout_bc = bass.AP(tensor=boutC.tensor, offset=boutC.offset,
                          ap=[[0, 128]] + [list(p) for p in boutC.ap])
        nc.sync.dma_start(out=bias_sb, in_=bout_bc)
        hoU = pat.tile([DH, 2, HC, TT], IN_DT, tag="hoU")
        hoT = pat.tile([DH, HC, TT], IN_DT, tag="hoT")
        dn = pat.tile([HC, 2, TT], F32, tag="dn")
        dnf = pat.tile([HC, 2, TT], F32R, tag="dnf")
        rcp = pat.tile([1, 2, HC, TT], F32R, tag="rcp")

        # finish work for the previous query tile is emitted piecewise
        # between score/AV groups so the PE fills exp-wait bubbles: 57
        # fine-grained pieces per query tile, len-paced across the next
        # tile's pop slots so most score groups get PE filler work.
        filler = []
        pace = [0]

        def pop_filler(force=False):
            if not filler:
                return
            pace[0] += len(filler)
            if force or pace[0] >= 64:
                pace[0] -= 64
                filler.pop(0)()

        def attn_unit(qt, h):
            """scores -> exp -> AV for (query tile qt, head h).

            Group-level software pipeline: sc(g) runs while exp(g-1) is on
            the scalar engine; av(g-1) follows sc(g)."""
            par = qt % 2
            qsl = slice(qt * TT, (qt + 1) * TT)
            ho_ps = psho.tile([DH + 1, TT], F32, tag="hops", name="hops")
            ex = [None, None]

            def sc_group(g):
                scp = psc.tile([128, NG, TT], F32, tag="scps", name="scps")
                for j in range(NG):
                    p = NG * g + j
                    nc.tensor.matmul(
                        out=scp[:, j, :],
                        lhsT=KTs[h][:, p * 128:(p + 1) * 128],
                        rhs=QTs[h][:, qsl], start=True, stop=True,
                    )
                ex[g % 2] = pex.tile([128, NG, TT], IN_DT, tag="ex", name="ex")
                nc.scalar.activation(out=ex[g % 2], in_=scp, func=AF.Exp,
                                     scale=SCALE)

            def av_group(g):
                for j in range(NG):
                    p = NG * g + j
                    nc.tensor.matmul(
                        out=ho_ps, lhsT=Vt[:, p, h, :], rhs=ex[g % 2][:, j, :],
                        start=(p == 0), stop=(p == NMT - 1),
                    )

            sc_group(0)
            for g in range(1, NGRP):
                sc_group(g)
                av_group(g - 1)
                pop_filler()
            av_group(NGRP - 1)
            # stash unnormalized numerator + denominator row (the
            # denominator goes via same-partition copy + DMA shuffle);
            # scalar handles the last tile so vector is free for the tail
            dt = pdt.tile([DH + 1, TT], F32, tag="dt", name="dt")
            if qt == NQT - 1:
                nc.scalar.copy(out=hoU[:, par, h, :], in_=ho_ps[0:DH, :])
                nc.scalar.copy(out=dt[DH:DH + 1, :], in_=ho_ps[DH:DH + 1, :])
            else:
                nc.vector.tensor_copy(out=hoU[:, par, h, :],
                                      in_=ho_ps[0:DH, :])
                nc.vector.tensor_copy(out=dt[DH:DH + 1, :],
                                      in_=ho_ps[DH:DH + 1, :])
            nc.sync.dma_start(out=dn[h:h + 1, par, :], in_=dt[DH:DH + 1, :])

        def fin_recip(qt):
            par = qt % 2
            with nc.allow_low_precision(reason="f32r is bitwise f32"):
                nc.vector.reciprocal(out=dnf[:, par, :], in_=dn[:, par, :])
            for h in range(HC):
                nc.sync.dma_start(out=rcp[0:1, par, h, :],
                                  in_=dnf[h:h + 1, par, :])

        def fin_norm(qt, h):
            par = qt % 2
            bc = psho.tile([DH + 1, TT], F32, tag="hops", name="bc")
            nc.tensor.matmul(out=bc[0:DH, :], lhsT=ones1,
                             rhs=rcp[0:1, par, h, :], start=True, stop=True)
            nc.vector.tensor_mul(out=hoT[:, h, :], in0=hoU[:, par, h, :],
                                 in1=bc[0:DH, :])

        def fin_chain_part(qt, tt_i, et, part, holder):
            """two matmuls of the 8-head out-proj chain; part 3 finishes."""
            if part == 0:
                holder[0] = pse.tile([128, TT], F32, tag="eps", name="eps")
            eps = holder[0]
            for h in (2 * part, 2 * part + 1):
                nc.tensor.matmul(
                    out=eps,
                    lhsT=hoT[:, h, tt_i * 128:(tt_i + 1) * 128],
                    rhs=WoSB[:, h, et * TT:(et + 1) * TT],
                    start=(h == 0), stop=(h == HC - 1),
                )
            if part == 3:
                osb = posb.tile([128, TT], F32, tag="osb", name="osb")
                nc.vector.tensor_add(
                    out=osb, in0=eps, in1=bias_sb[:, et * TT:(et + 1) * TT]
                )
                nc.sync.dma_start(out=out[(qt * 4 + tt_i) * 3 + et], in_=osb)

        def queue_finish(qt):
            filler.append(lambda qt=qt: fin_recip(qt))
            for h in range(HC):
                filler.append(lambda qt=qt, h=h: fin_norm(qt, h))
            for tt_i in range(TT // 128):
                for et in range(DIM // TT):
                    holder = [None]
                    for part in range(4):
                        filler.append(
                            lambda qt=qt, t=tt_i, e=et, p=part, hd=holder:
                            fin_chain_part(qt, t, e, p, hd))

        for qt in range(NQT):
            for h in range(HC):
                attn_unit(qt, h)
                pop_filler()
            queue_finish(qt)
        while filler:
            pop_filler(force=True)


def build():
    from contextlib import ExitStack

    nc = bacc.Bacc("TRN2", target_bir_lowering=False, debug=False)
    xP_p = nc.declare_dram_parameter("xP", [128, NMT, KT, 128], IN_DT,
                                     isOutput=False)
    fN_p = nc.declare_dram_parameter("fN", [128, NMT, DH], F32, isOutput=False)
    id_p = nc.declare_dram_parameter("ident", [128, 128], IN_DT, isOutput=False)
    Wall_p = nc.declare_dram_parameter("Wall", [128, 6, KT, 384], IN_DT,
                                       isOutput=False)
    WoC_p = nc.declare_dram_parameter("WoC", [HD, DIM], IN_DT, isOutput=False)
    bout_p = nc.declare_dram_parameter("boutC", [DIM], F32, isOutput=False)
    out = nc.declare_dram_parameter("out", [NQT * 4 * 3, 128, TT], F32,
                                    isOutput=True)
    io = tuple(
        t[:] for t in (xP_p, fN_p, id_p, Wall_p, WoC_p, bout_p, out)
    )
    with ExitStack() as ctx:
        tc = ctx.enter_context(tile.TileContext(nc))
        _emit(ctx, tc, io)
    nc.finalize()
    return nc


def make_in_maps(x, f1, f2, f3, Wqkv, Wout, bout):
    x = np.asarray(x, np.float32)
    fcat = np.concatenate(
        [np.asarray(f1, np.float32), np.asarray(f2, np.float32),
         np.asarray(f3, np.float32)], axis=1,
    )  # [N, DH]
    # fN packed to [p, t, c]: fN[p,t,c] = fcat[t*128+p, c]
    fN_np = np.ascontiguousarray(
        fcat.reshape(NMT, 128, DH).transpose(1, 0, 2))
    ident_np = np.eye(128, dtype=np.float32).astype(IN_NP)
    Wqkv = np.asarray(Wqkv, np.float32)
    Wout = np.asarray(Wout, np.float32)
    bout = np.ascontiguousarray(np.asarray(bout, np.float32))
    zbias = np.zeros_like(bout)
    # x packed to [p, t, k, n]: xP[p,t,k,n] = x[b, t*128+n, k*128+p]
    xPs = [np.ascontiguousarray(
        x[b].reshape(NMT, 128, KT, 128).transpose(3, 0, 2, 1)).astype(IN_NP)
        for b in range(B)]
    Wslice = []
    for hh in range(2):
        cs = hh * HD
        wall = np.concatenate(
            [Wqkv[:, cs:cs + HD],
             Wqkv[:, H * DH + cs:H * DH + cs + HD],
             Wqkv[:, 2 * H * DH + cs:2 * H * DH + cs + HD]], axis=1,
        )  # [DIM, 2304]
        # packed to [p, c, k, j]: Wall[p,c,k,j] = wall[k*128+p, c*384+j]
        wall_p = np.ascontiguousarray(
            wall.reshape(KT, 128, 6, 384).transpose(1, 2, 0, 3)).astype(IN_NP)
        Wslice.append(dict(
            Wall=wall_p,
            WoC=np.ascontiguousarray(Wout[cs:cs + HD, :]).astype(IN_NP),
        ))
    in_maps = []
    for c in range(8):
        b, hh = divmod(c, 2)
        in_maps.append(dict(
            xP=xPs[b], fN=fN_np, ident=ident_np,
            boutC=bout if hh == 0 else zbias,
            **Wslice[hh],
        ))
    return in_maps


_NC_CACHE = None


def kernel(x, f1, f2, f3, Wqkv, Wout, bout, _trace=False):
    global _NC_CACHE
    if _NC_CACHE is None:
        _NC_CACHE = build()
    nc = _NC_CACHE
    in_maps = make_in_maps(x, f1, f2, f3, Wqkv, Wout, bout)
    res = run_bass_kernel_spmd(nc, in_maps, list(range(8)), trace=_trace)
    out = np.empty((B, N, DIM), np.float32)
    for b in range(B):
        blk = res.results[2 * b]["out"] + res.results[2 * b + 1]["out"]
        # [qt*4+tt, et, 128, 512] blocks -> [2048, 1536]
        out[b] = (blk.reshape(NMT, 3, 128, TT).transpose(0, 2, 1, 3)
                  .reshape(N, DIM))
    if _trace:
        return out, res
    return out
